# revision 1
# baseline (speedup 1.0000x reference)
"""Trainium2 Bass kernel for nn_AD_F_56384330662393 (dense_cnn, 3-iter
diffusion).  Data-parallel over batch: 32 images -> 8 cores x 4 images.

Scheme C ("q-folded im2col"): per image, 16 blocks of 32 output rows.
Per block ONE PSUM accumulation group of 4 matmuls:
  - A: lhsT [108,128] (3 col-shifts x 36 rows folded into K), all 4 conv
    channels packed into M=128 (out partition = 32k + r)
  - B: lhsT [72,128] (2 more col-shifts)
  - corrL/corrR: N=1 column-boundary corrections.
The rhs operands are DMA-gathered replicas (xrep) of 36-row x windows,
5 column-shifted copies each, built with 2-3 dim strided SBUF->SBUF DMAs.
PReLU^2 via one ScalarE Prelu over 2 banks then DVE square; channel
reduction S = sum_k p_k^2 via 4 col-tiled 0/1 matmuls (tile_position)
into one PSUM bank; one DVE sub per 128-row column-tile updates x.

x layout: [128, im, v, 516] bf16 — image row r = (partition r%128,
col-tile v = r//128), 2 zero ghost cols each side.  No ghost rows; all
halos come from the gather patches.  x/weights/output shipped bf16.
"""
import numpy as np
import ml_dtypes

SHIFTS = [(-1, 0), (1, 0), (0, -1), (0, 1), (-1, -1), (-1, 1), (1, -1), (1, 1)]
H = 512
NK = 4            # conv output channels
NT = 3            # iterations
NIMG = 4          # images per core
NCORES = 8
BLK = 32          # out rows per block
RHO = 36          # input rows per block window
KA = 3 * RHO      # pass-A contraction (q = -2..0)
KB = 2 * RHO      # pass-B contraction (q = +1..+2)
PP = NIMG * 4 * 516          # xb elements per partition
XP = 4 * NIMG * 4 * 516      # xrep elements per partition (u, im, v, 516)
SPAN = PP - 4                # merged (im,v,c) gather span
VS = 1544                    # patch gather span (3 v-blocks through c=511)


def _build_C(Wt, R_rows):
    """Effective 5x5 stencil with row masks. Wt [4,8,3,3] float64."""
    nR = len(R_rows)
    C = np.zeros((NK, 5, 5, nR))
    CL = np.zeros((NK, 5, nR))
    CR = np.zeros((NK, 5, nR))
    R = np.asarray(R_rows)
    for c, (di, dj) in enumerate(SHIFTS):
        for u in (-1, 0, 1):
            p = u + di
            m = ((R + u >= 0) & (R + u < H) & (R + p >= 0) & (R + p < H))
            for v in (-1, 0, 1):
                q = v + dj
                w = Wt[:, c, u + 1, v + 1]
                C[:, p + 2, q + 2, :] += w[:, None] * m[None, :]
                if q == 0 and (v, dj) == (-1, 1):
                    CL[:, p + 2, :] -= w[:, None] * m[None, :]
                if q == 0 and (v, dj) == (1, -1):
                    CR[:, p + 2, :] -= w[:, None] * m[None, :]
    return C, CL, CR


def _build_lhsT(Wt, b):
    """lhsT_A [108,128], lhsT_B [72,128], corrL/corrR [108,128] for block b."""
    C, CL, CR = _build_C(Wt, [BLK * b + r for r in range(BLK)])
    A = np.zeros((KA, 128))
    Bm = np.zeros((KB, 128))
    cl = np.zeros((KA, 128))
    cr = np.zeros((KA, 128))
    for r in range(BLK):
        for rho in range(RHO):
            p = rho - 2 - r
            if not -2 <= p <= 2:
                continue
            for k in range(NK):
                for qa in range(3):
                    A[qa * RHO + rho, 32 * k + r] = C[k, p + 2, qa, r]
                for qb in range(2):
                    Bm[qb * RHO + rho, 32 * k + r] = C[k, p + 2, qb + 3, r]
                cl[2 * RHO + rho, 32 * k + r] = CL[k, p + 2, r]
                cr[2 * RHO + rho, 32 * k + r] = CR[k, p + 2, r]
    return A, Bm, cl, cr


def _gather(nc, AP, xb, xrep, nsh, coff, pair):
    """Gathers for one image pair (im = 2*pair, 2*pair+1):
    xrep[36q+rho, u, im, v, c] = xb[32u+rho-2, im, v, c+coff+q]."""
    poff = pair * 2 * 2064          # im offset within a partition row
    for q in range(nsh):
        for u in range(4):
            r0 = 2 if u == 0 else 0
            r1 = 36 if u < 3 else 34
            nrho = r1 - r0
            src = AP(xb.tensor,
                     xb.offset + (32 * u + r0 - 2) * PP + poff + coff + q,
                     [[PP, nrho], [1, 2 * 2064 - 4]])
            dst = AP(xrep.tensor,
                     xrep.offset + (36 * q + r0) * XP + u * PP + poff,
                     [[XP, nrho], [1, 2 * 2064 - 4]])
            nc.sync.dma_start(out=dst, in_=src)


def _patches(nc, AP, xb, xrep, nsh, coff):
    """Cross-tile halo patches (all images in one DMA per (q, side))."""
    for q in range(nsh):
        # u=0, rho {0,1} <- prev tile partitions 126,127 (v' = v-1: 0..2)
        src = AP(xb.tensor, xb.offset + 126 * PP + coff + q,
                 [[PP, 2], [2064, NIMG], [1, VS]])
        dst = AP(xrep.tensor, xrep.offset + (36 * q) * XP + 0 * PP + 516,
                 [[XP, 2], [2064, NIMG], [1, VS]])
        nc.sync.dma_start(out=dst, in_=src)
        # u=3, rho {34,35} <- next tile partitions 0,1 (v' = v+1: 1..3)
        src = AP(xb.tensor, xb.offset + 0 * PP + 516 + coff + q,
                 [[PP, 2], [2064, NIMG], [1, VS]])
        dst = AP(xrep.tensor,
                 xrep.offset + (36 * q + 34) * XP + 3 * PP + 0 * 516,
                 [[XP, 2], [2064, NIMG], [1, VS]])
        nc.sync.dma_start(out=dst, in_=src)


def _build_graph(bf, af):
    from contextlib import ExitStack
    import concourse.bass as bass
    import concourse.tile as tile
    from concourse import mybir
    from concourse.ap import AP

    nc = bass.Bass()
    bf16 = mybir.dt.bfloat16
    f32 = mybir.dt.float32
    LR = mybir.ActivationFunctionType.Prelu

    x_ext = nc.declare_dram_parameter("x", [128, NIMG, 4, 516], bf16,
                                      isOutput=False)
    # weights packed: [128, NT, 3var, 4 kinds(A,B,cl,cr), 128] bf16
    w_ext = nc.declare_dram_parameter("wts", [128, NT, 3, 4, 128], bf16,
                                      isOutput=False)
    red_ext = nc.declare_dram_parameter("red", [128, 32], bf16, isOutput=False)
    bias_ext = nc.declare_dram_parameter("biasv", [128, NT], f32,
                                         isOutput=False)
    out_ext = nc.declare_dram_parameter("out", [128, NIMG, 4, 516], bf16,
                                        isOutput=True)

    use_bias = bool(np.any(bf))

    with tile.TileContext(nc) as tc:
        with ExitStack() as ctx:
            persist = ctx.enter_context(tc.tile_pool(name="persist", bufs=1))
            ppool = ctx.enter_context(tc.tile_pool(name="pt", bufs=3))
            p2pool = ctx.enter_context(tc.tile_pool(name="p2", bufs=2))
            pacc_pool = ctx.enter_context(
                tc.tile_pool(name="pa", bufs=3, space="PSUM"))
            sacc_pool = ctx.enter_context(
                tc.tile_pool(name="sa", bufs=2, space="PSUM"))

            wts = persist.tile([128, NT, 3, 4, 128], bf16, tag="wts")
            red = persist.tile([128, 32], bf16, tag="red")
            biasv = persist.tile([128, NT], f32, tag="biasv")
            nc.sync.dma_start(out=wts, in_=w_ext[:, :, :, :, :])
            nc.sync.dma_start(out=red, in_=red_ext[:, :])
            nc.sync.dma_start(out=biasv, in_=bias_ext[:, :])

            xb = persist.tile([128, NIMG, 4, 516], bf16, tag="xb")
            xrepA = persist.tile([KA, 4, NIMG, 4, 516], bf16, tag="xrepA")
            xrepB = persist.tile([KB, 4, NIMG, 4, 516], bf16, tag="xrepB")
            nc.vector.memset(xrepA, 0)
            nc.vector.memset(xrepB, 0)
            for pair in (0, 1):
                nc.sync.dma_start(
                    out=xb[:, 2 * pair:2 * pair + 2, :, :],
                    in_=x_ext[:, 2 * pair:2 * pair + 2, :, :])

            for t in range(NT):
                alpha = float(np.sqrt(af[t]))
                for pair in (0, 1):
                    _gather(nc, AP, xb, xrepA, 3, 0, pair)
                    _gather(nc, AP, xb, xrepB, 2, 3, pair)
                _patches(nc, AP, xb, xrepA, 3, 0)
                _patches(nc, AP, xb, xrepB, 2, 3)

                for im in range(NIMG):
                    for v in range(4):
                        p2 = p2pool.tile([128, 4, 512], bf16, tag="p2")
                        for pairu in (0, 1):
                            acc = pacc_pool.tile([128, 2, 512], mybir.dt.float32,
                                                 tag="acc")
                            for j in (0, 1):
                                u = 2 * pairu + j
                                var = 0 if (v == 0 and u == 0) else \
                                    (2 if (v == 3 and u == 3) else 1)
                                nc.tensor.matmul(
                                    acc[:, j, :], wts[0:KA, t, var, 0, :],
                                    xrepA[:, u, im, v, 0:512],
                                    start=True, stop=False)
                                nc.tensor.matmul(
                                    acc[:, j, :], wts[0:KB, t, var, 1, :],
                                    xrepB[:, u, im, v, 0:512],
                                    start=False, stop=False)
                                nc.tensor.matmul(
                                    acc[:, j, 0:1], wts[0:KA, t, var, 2, :],
                                    xrepA[:, u, im, v, 0:1],
                                    start=False, stop=False)
                                nc.tensor.matmul(
                                    acc[:, j, 511:512], wts[0:KA, t, var, 3, :],
                                    xrepA[:, u, im, v, 511:512],
                                    start=False, stop=True)
                            ptile = ppool.tile([128, 2, 512], bf16, tag="ptile")
                            if use_bias:
                                nc.scalar.activation(
                                    out=ptile, in_=acc, func=LR,
                                    bias=biasv[:, t:t + 1], scale=0.5,
                                    alpha=alpha)
                            else:
                                nc.scalar.activation(
                                    out=ptile, in_=acc, func=LR,
                                    bias=0.0, scale=0.5, alpha=alpha)
                            nc.vector.tensor_mul(
                                p2[:, 2 * pairu:2 * pairu + 2, :],
                                ptile, ptile)
                        sacc = sacc_pool.tile([128, 512], mybir.dt.float32,
                                              tag="sacc")
                        for u in range(4):
                            nc.tensor.matmul(
                                sacc[32 * u:32 * u + 32, :], red, p2[:, u, :],
                                start=True, stop=True,
                                tile_position=(0, 32 * u))
                        nc.vector.tensor_sub(
                            xb[:, im, v, 2:514], xb[:, im, v, 2:514], sacc)

            for pair in (0, 1):
                nc.sync.dma_start(
                    out=out_ext[:, 2 * pair:2 * pair + 2, :, :],
                    in_=xb[:, 2 * pair:2 * pair + 2, :, :])

    _split_multiwait_drains(nc)
    return nc


def _split_multiwait_drains(nc):
    """Walrus workaround: this compiler build only accepts one sem-wait per
    instruction; peel extras onto injected same-engine NoOps placed just
    before (engine streams run in program order, so semantics are equal)."""
    from concourse import mybir
    import bass_rust

    for f in nc.m.functions:
        for bb in f.blocks:
            idx = 0
            while idx < len(bb.instructions):
                inst = bb.instructions[idx]
                si = getattr(inst, "sync_info", None)
                if si is not None and si.on_wait and len(si.on_wait) > 1:
                    waits = list(si.on_wait)
                    upd = list(si.on_update) if si.on_update else []
                    for j, w in enumerate(waits[:-1]):
                        nop = mybir.InstNoOp(
                            name=f"{inst.name}-wsplit{j}", ins=[], outs=[])
                        nop.engine = inst.engine
                        nop.sync_info = bass_rust.SyncInfo(
                            on_wait=[w], on_update=[])
                        nc.register_instruction(nop, overwrite=True)
                        bb.instructions.insert(idx, nop)
                        idx += 1
                    inst.sync_info = bass_rust.SyncInfo(
                        on_wait=[waits[-1]], on_update=upd)
                idx += 1


def kernel(x, W, b, a):
    from concourse.bass_utils import run_bass_kernel_spmd

    x = np.asarray(x)
    Wf = np.asarray(W, dtype=np.float64)
    bfv = np.asarray(b, dtype=np.float64)
    af = np.asarray(a, dtype=np.float64)

    # weights: [128, NT, 3var, 4kinds, 128] (partition = contraction index)
    wts = np.zeros((128, NT, 3, 4, 128), np.float64)
    for t in range(NT):
        for vi, blk in enumerate((0, 1, 15)):
            A, Bm, cl, cr = _build_lhsT(Wf[t], blk)
            wts[0:KA, t, vi, 0, :] = A
            wts[0:KB, t, vi, 1, :] = Bm
            wts[0:KA, t, vi, 2, :] = cl
            wts[0:KA, t, vi, 3, :] = cr
    wts = wts.astype(ml_dtypes.bfloat16)

    red = np.zeros((128, 32), ml_dtypes.bfloat16)
    for k in range(NK):
        for r in range(BLK):
            red[32 * k + r, r] = 1.0
    biasv = np.zeros((128, NT), np.float32)
    for t in range(NT):
        biasv[:, t] = np.repeat(0.5 * bfv[t], BLK)

    nc = _build_graph(bfv, af)

    xall = x[:, 0].astype(ml_dtypes.bfloat16)   # [32, 512, 512]
    in_maps = []
    for core in range(NCORES):
        shard = xall[core * NIMG:(core + 1) * NIMG]
        xt = np.zeros((128, NIMG, 4, 516), dtype=ml_dtypes.bfloat16)
        for v in range(4):
            xt[:, :, v, 2:514] = shard[:, 128 * v:128 * v + 128, :] \
                .transpose(1, 0, 2)
        in_maps.append({"x": xt, "wts": wts, "red": red, "biasv": biasv})
    res = run_bass_kernel_spmd(nc, in_maps, list(range(NCORES)))
    global LAST_RESULT
    LAST_RESULT = res
    out = np.empty((32, H, 512), dtype=np.float32)
    for core in range(NCORES):
        ot = np.asarray(res.results[core]["out"], dtype=np.float32)
        for v in range(4):
            out[core * NIMG:(core + 1) * NIMG, 128 * v:128 * v + 128, :] = \
                ot[:, :, v, 2:514].transpose(1, 0, 2)
    return out[:, None, :, :].astype(x.dtype)


LAST_RESULT = None



# revision 9
# speedup vs baseline: 1.0174x; 1.0174x over previous
"""Trainium2 Bass kernel for nn_AD_F_56384330662393 (dense_cnn, 3-iter
diffusion).  Data-parallel over batch: 32 images -> 8 cores x 4 images.

Scheme C ("q-folded im2col") with DRAM-routed halo gathers:
per image, 16 blocks of 32 output rows.  Per block ONE PSUM accumulation
group of 4 matmuls:
  - A: lhsT [108,128] (3 col-shifts x 36 rows folded into K), all 4 conv
    channels packed into M=128 (out partition = 32k + r)
  - B: lhsT [72,128] (2 more col-shifts)
  - corrL/corrR: N=1 column-boundary corrections (feature-pad semantics).
The rhs operands are q-replicated 36-row windows (xrep), gathered from a
row-linear DRAM scratch (x rows interleaved over images at stride 516,
2 ghost rows/cols zeroed host-side).  DRAM-side strides absorb the
(q: +1 col, rho: +1 row) pattern, so one DMA per (kind,u,v) rebuilds a
whole 108/72-partition slab and the descriptors spread across all 16
SDMA engines (SBUF->SBUF gathers collapse onto 1 engine pair; DRAM
round-trip is ~3x faster end-to-end).

Pipelined v-major loop: after tile v's 4 images are updated, xb[:,v] is
written back to the per-v scratch (plus 2-row halo strips to v+-1) and
the iteration-(t+1) gathers for tile v-1 are issued immediately, so all
gather/writeback DMA runs in the shadow of the next tiles' matmuls.
Gathers/writebacks alternate between the two HWDGE rings (sync/scalar).

PReLU^2 via one ScalarE Prelu over 2 banks then DVE square; channel
reduction S = sum_k p_k^2 via 4 col-tiled 0/1 matmuls (tile_position)
into one PSUM bank; one DVE sub per 128-row column-tile updates x.

x layout: [128, v, im, 516] bf16 in SBUF (image row r = partition r%128,
col-tile v = r//128); scratch_v in DRAM: [132, im, 516] rows 128v-2 ..
128v+129.  x/weights/output shipped bf16.
"""
import numpy as np
import ml_dtypes

SHIFTS = [(-1, 0), (1, 0), (0, -1), (0, 1), (-1, -1), (-1, 1), (1, -1), (1, 1)]
H = 512
NK = 4            # conv output channels
NT = 3            # iterations
NIMG = 4          # images per core
NCORES = 8
BLK = 32          # out rows per block
RHO = 36          # input rows per block window
KA = 3 * RHO      # pass-A contraction (q = -2..0)
KB = 2 * RHO      # pass-B contraction (q = +1..+2)
ROW = NIMG * 516              # one scratch row: 4 images interleaved
PP = 4 * ROW                  # xb elements per partition (v, im, 516)
XP = 4 * PP                   # xrep elements per partition (u, v, im, 516)
SCR_ROWS = 133                # 128 + 2 ghost rows each side + 1 pad
                              # (q-shifted gather reads a few elements
                              #  past row 131; pad row stays zero)


def _build_C(Wt, R_rows):
    """Effective 5x5 stencil with row masks. Wt [4,8,3,3] float64."""
    nR = len(R_rows)
    C = np.zeros((NK, 5, 5, nR))
    CL = np.zeros((NK, 5, nR))
    CR = np.zeros((NK, 5, nR))
    R = np.asarray(R_rows)
    for c, (di, dj) in enumerate(SHIFTS):
        for u in (-1, 0, 1):
            p = u + di
            m = ((R + u >= 0) & (R + u < H) & (R + p >= 0) & (R + p < H))
            for v in (-1, 0, 1):
                q = v + dj
                w = Wt[:, c, u + 1, v + 1]
                C[:, p + 2, q + 2, :] += w[:, None] * m[None, :]
                if q == 0 and (v, dj) == (-1, 1):
                    CL[:, p + 2, :] -= w[:, None] * m[None, :]
                if q == 0 and (v, dj) == (1, -1):
                    CR[:, p + 2, :] -= w[:, None] * m[None, :]
    return C, CL, CR


def _build_lhsT(Wt, b):
    """lhsT_A [108,128], lhsT_B [72,128], corrL/corrR [108,128] for block b."""
    C, CL, CR = _build_C(Wt, [BLK * b + r for r in range(BLK)])
    A = np.zeros((KA, 128))
    Bm = np.zeros((KB, 128))
    cl = np.zeros((KA, 128))
    cr = np.zeros((KA, 128))
    for r in range(BLK):
        for rho in range(RHO):
            p = rho - 2 - r
            if not -2 <= p <= 2:
                continue
            for k in range(NK):
                for qa in range(3):
                    A[qa * RHO + rho, 32 * k + r] = C[k, p + 2, qa, r]
                for qb in range(2):
                    Bm[qb * RHO + rho, 32 * k + r] = C[k, p + 2, qb + 3, r]
                cl[2 * RHO + rho, 32 * k + r] = CL[k, p + 2, r]
                cr[2 * RHO + rho, 32 * k + r] = CR[k, p + 2, r]
    return A, Bm, cl, cr


def _build_graph(bf, af):
    from contextlib import ExitStack
    import concourse.bass as bass
    import concourse.tile as tile
    from concourse import mybir
    from concourse.ap import AP

    nc = bass.Bass()
    bf16 = mybir.dt.bfloat16
    f32 = mybir.dt.float32
    LR = mybir.ActivationFunctionType.Prelu

    scrs = [nc.declare_dram_parameter(f"xscr{v}", [SCR_ROWS, NIMG, 516],
                                      bf16, isOutput=False)
            for v in range(4)]
    # weights packed: [128, NT, 3var, 4 kinds(A,B,cl,cr), 128] bf16
    w_ext = nc.declare_dram_parameter("wts", [128, NT, 3, 4, 128], bf16,
                                      isOutput=False)
    red_ext = nc.declare_dram_parameter("red", [128, 32], bf16, isOutput=False)
    bias_ext = nc.declare_dram_parameter("biasv", [128, NT], f32,
                                         isOutput=False)
    out_ext = nc.declare_dram_parameter("out", [4, 128, NIMG, 516], bf16,
                                        isOutput=True)

    use_bias = bool(np.any(bf))
    rings = [nc.sync, nc.scalar]
    ring_i = [0]

    def ring():
        ring_i[0] ^= 1
        return rings[ring_i[0]]

    with tile.TileContext(nc) as tc:
        with ExitStack() as ctx:
            persist = ctx.enter_context(tc.tile_pool(name="persist", bufs=1))
            ppool = ctx.enter_context(tc.tile_pool(name="pt", bufs=3))
            p2pool = ctx.enter_context(tc.tile_pool(name="p2", bufs=2))
            pacc_pool = ctx.enter_context(
                tc.tile_pool(name="pa", bufs=3, space="PSUM"))
            sacc_pool = ctx.enter_context(
                tc.tile_pool(name="sa", bufs=2, space="PSUM"))

            wts = persist.tile([128, NT, 3, 4, 128], bf16, tag="wts")
            red = persist.tile([128, 32], bf16, tag="red")
            biasv = persist.tile([128, NT], f32, tag="biasv")
            nc.sync.dma_start(out=wts, in_=w_ext[:, :, :, :, :])
            nc.sync.dma_start(out=red, in_=red_ext[:, :])
            nc.sync.dma_start(out=biasv, in_=bias_ext[:, :])

            xb = persist.tile([128, 4, NIMG, 516], bf16, tag="xb")
            xrepA = persist.tile([KA, 4, 4, NIMG, 516], bf16, tag="xrepA")
            xrepB = persist.tile([KB, 4, 4, NIMG, 516], bf16, tag="xrepB")

            def gather(v):
                """xrep[(q,rho), u, v, im, cg] = scr_v[32u+rho, im, cg+coff+q]
                for both kinds; one DMA per (kind, u)."""
                for u in range(4):
                    for xrep, nsh, coff in ((xrepA, 3, 0), (xrepB, 2, 3)):
                        src = AP(scrs[v], (32 * u) * ROW + coff,
                                 [[1, nsh], [ROW, RHO], [1, ROW]])
                        dst = AP(xrep.tensor,
                                 xrep.offset + u * PP + v * ROW,
                                 [[XP, nsh * RHO], [1, ROW]])
                        ring().dma_start(out=dst, in_=src)

            def writeback(v, t):
                src = AP(xb.tensor, xb.offset + v * ROW, [[PP, 128], [1, ROW]])
                if t == NT - 1:
                    dst = AP(out_ext, v * 128 * ROW, [[ROW, 128], [1, ROW]])
                    ring().dma_start(out=dst, in_=src)
                    return
                dst = AP(scrs[v], 2 * ROW, [[ROW, 128], [1, ROW]])
                ring().dma_start(out=dst, in_=src)
                if v > 0:     # rows 128v..128v+1 -> scr_{v-1}[130:132]
                    s = AP(xb.tensor, xb.offset + v * ROW, [[PP, 2], [1, ROW]])
                    d = AP(scrs[v - 1], 130 * ROW, [[ROW, 2], [1, ROW]])
                    ring().dma_start(out=d, in_=s)
                if v < 3:     # rows 128v+126..127 -> scr_{v+1}[0:2]
                    s = AP(xb.tensor, xb.offset + 126 * PP + v * ROW,
                           [[PP, 2], [1, ROW]])
                    d = AP(scrs[v + 1], 0, [[ROW, 2], [1, ROW]])
                    ring().dma_start(out=d, in_=s)

            # prologue: t=0 gathers + xb load, all from host-filled scratch
            for v in range(4):
                gather(v)
                src = AP(scrs[v], 2 * ROW, [[ROW, 128], [1, ROW]])
                dst = AP(xb.tensor, xb.offset + v * ROW, [[PP, 128], [1, ROW]])
                ring().dma_start(out=dst, in_=src)

            for t in range(NT):
                alpha = float(np.sqrt(af[t]))
                for v in range(4):
                    for im in range(NIMG):
                        p2 = p2pool.tile([128, 4, 512], bf16, tag="p2")
                        for pairu in (0, 1):
                            acc = pacc_pool.tile([128, 2, 512],
                                                 mybir.dt.float32, tag="acc")
                            for j in (0, 1):
                                u = 2 * pairu + j
                                var = 0 if (v == 0 and u == 0) else \
                                    (2 if (v == 3 and u == 3) else 1)
                                nc.tensor.matmul(
                                    acc[:, j, :], wts[0:KA, t, var, 0, :],
                                    xrepA[:, u, v, im, 0:512],
                                    start=True, stop=False)
                                nc.tensor.matmul(
                                    acc[:, j, :], wts[0:KB, t, var, 1, :],
                                    xrepB[:, u, v, im, 0:512],
                                    start=False, stop=False)
                                nc.tensor.matmul(
                                    acc[:, j, 0:1], wts[0:KA, t, var, 2, :],
                                    xrepA[:, u, v, im, 0:1],
                                    start=False, stop=False)
                                nc.tensor.matmul(
                                    acc[:, j, 511:512], wts[0:KA, t, var, 3, :],
                                    xrepA[:, u, v, im, 511:512],
                                    start=False, stop=True)
                            ptile = ppool.tile([128, 2, 512], bf16, tag="ptile")
                            if use_bias:
                                nc.scalar.activation(
                                    out=ptile, in_=acc, func=LR,
                                    bias=biasv[:, t:t + 1], scale=0.5,
                                    alpha=alpha)
                            else:
                                nc.scalar.activation(
                                    out=ptile, in_=acc, func=LR,
                                    bias=0.0, scale=0.5, alpha=alpha)
                            nc.vector.tensor_mul(
                                p2[:, 2 * pairu:2 * pairu + 2, :],
                                ptile, ptile)
                        sacc = sacc_pool.tile([128, 512], mybir.dt.float32,
                                              tag="sacc")
                        for u in range(4):
                            nc.tensor.matmul(
                                sacc[32 * u:32 * u + 32, :], red, p2[:, u, :],
                                start=True, stop=True,
                                tile_position=(0, 32 * u))
                        nc.vector.tensor_sub(
                            xb[:, v, im, 2:514], xb[:, v, im, 2:514], sacc)
                    writeback(v, t)
                    if t < NT - 1:
                        if v >= 1:
                            gather(v - 1)
                        if v == 3:
                            gather(3)

    _split_multiwait_drains(nc)
    return nc


def _split_multiwait_drains(nc):
    """Walrus workaround: this compiler build only accepts one sem-wait per
    instruction; peel extras onto injected same-engine NoOps placed just
    before (engine streams run in program order, so semantics are equal)."""
    from concourse import mybir
    import bass_rust

    for f in nc.m.functions:
        for bb in f.blocks:
            idx = 0
            while idx < len(bb.instructions):
                inst = bb.instructions[idx]
                si = getattr(inst, "sync_info", None)
                if si is not None and si.on_wait and len(si.on_wait) > 1:
                    waits = list(si.on_wait)
                    upd = list(si.on_update) if si.on_update else []
                    for j, w in enumerate(waits[:-1]):
                        nop = mybir.InstNoOp(
                            name=f"{inst.name}-wsplit{j}", ins=[], outs=[])
                        nop.engine = inst.engine
                        nop.sync_info = bass_rust.SyncInfo(
                            on_wait=[w], on_update=[])
                        nc.register_instruction(nop, overwrite=True)
                        bb.instructions.insert(idx, nop)
                        idx += 1
                    inst.sync_info = bass_rust.SyncInfo(
                        on_wait=[waits[-1]], on_update=upd)
                idx += 1


def kernel(x, W, b, a):
    from concourse.bass_utils import run_bass_kernel_spmd

    x = np.asarray(x)
    Wf = np.asarray(W, dtype=np.float64)
    bfv = np.asarray(b, dtype=np.float64)
    af = np.asarray(a, dtype=np.float64)

    # weights: [128, NT, 3var, 4kinds, 128] (partition = contraction index)
    wts = np.zeros((128, NT, 3, 4, 128), np.float64)
    for t in range(NT):
        for vi, blk in enumerate((0, 1, 15)):
            A, Bm, cl, cr = _build_lhsT(Wf[t], blk)
            wts[0:KA, t, vi, 0, :] = A
            wts[0:KB, t, vi, 1, :] = Bm
            wts[0:KA, t, vi, 2, :] = cl
            wts[0:KA, t, vi, 3, :] = cr
    wts = wts.astype(ml_dtypes.bfloat16)

    red = np.zeros((128, 32), ml_dtypes.bfloat16)
    for k in range(NK):
        for r in range(BLK):
            red[32 * k + r, r] = 1.0
    biasv = np.zeros((128, NT), np.float32)
    for t in range(NT):
        biasv[:, t] = np.repeat(0.5 * bfv[t], BLK)

    nc = _build_graph(bfv, af)

    xall = x[:, 0].astype(ml_dtypes.bfloat16)   # [32, 512, 512]
    in_maps = []
    for core in range(NCORES):
        shard = xall[core * NIMG:(core + 1) * NIMG]   # [4, 512, 512]
        im = {"wts": wts, "red": red, "biasv": biasv}
        for v in range(4):
            scr = np.zeros((SCR_ROWS, NIMG, 516), dtype=ml_dtypes.bfloat16)
            lo = 2 if v == 0 else 0
            hi = 130 if v == 3 else 132
            scr[lo:hi, :, 2:514] = \
                shard[:, 128 * v - 2 + lo:128 * v - 2 + hi, :] \
                .transpose(1, 0, 2)
            im[f"xscr{v}"] = scr
        in_maps.append(im)
    res = run_bass_kernel_spmd(nc, in_maps, list(range(NCORES)))
    global LAST_RESULT
    LAST_RESULT = res
    out = np.empty((32, H, 512), dtype=np.float32)
    for core in range(NCORES):
        ot = np.asarray(res.results[core]["out"], dtype=np.float32)
        for v in range(4):
            out[core * NIMG:(core + 1) * NIMG, 128 * v:128 * v + 128, :] = \
                ot[v, :, :, 2:514].transpose(1, 0, 2)
    return out[:, None, :, :].astype(x.dtype)


LAST_RESULT = None


# revision 13
# speedup vs baseline: 1.7974x; 1.7667x over previous
"""Trainium2 Bass kernel for nn_AD_F_56384330662393 (dense_cnn, 3-iter
diffusion).  Data-parallel over batch: 32 images -> 8 cores x 4 images.

Scheme C ("q-folded im2col") with DRAM-routed halo gathers:
per image, 16 blocks of 32 output rows.  Per block ONE PSUM accumulation
group of 4 matmuls:
  - A: lhsT [108,128] (3 col-shifts x 36 rows folded into K), all 4 conv
    channels packed into M=128 (out partition = 32k + r)
  - B: lhsT [72,128] (2 more col-shifts)
  - corrL/corrR: N=1 column-boundary corrections (feature-pad semantics).
The rhs operands are q-replicated 36-row windows (xrep), gathered from a
row-linear DRAM scratch (x rows interleaved over images at stride 516,
2 ghost rows/cols zeroed host-side).  DRAM-side strides absorb the
(q: +1 col, rho: +1 row) pattern, so one DMA per (kind,u,v) rebuilds a
whole 108/72-partition slab and the descriptors spread across all 16
SDMA engines (SBUF->SBUF gathers collapse onto 1 engine pair; DRAM
round-trip is ~3x faster end-to-end).

Pipelined v-major loop: after tile v's 4 images are updated, xb[:,v] is
written back to the per-v scratch (plus 2-row halo strips to v+-1) and
the iteration-(t+1) gathers for tile v-1 are issued immediately, so all
gather/writeback DMA runs in the shadow of the next tiles' matmuls.
Gathers/writebacks alternate between the two HWDGE rings (sync/scalar).

PReLU^2 via one ScalarE Prelu over 2 banks then DVE square; channel
reduction S = sum_k p_k^2 via 4 col-tiled 0/1 matmuls (tile_position)
into one PSUM bank; one DVE sub per 128-row column-tile updates x.

x layout: [128, v, im, 516] bf16 in SBUF (image row r = partition r%128,
col-tile v = r//128); scratch_v in DRAM: [132, im, 516] rows 128v-2 ..
128v+129.  x/weights/output shipped bf16.
"""
import numpy as np
import ml_dtypes

SHIFTS = [(-1, 0), (1, 0), (0, -1), (0, 1), (-1, -1), (-1, 1), (1, -1), (1, 1)]
H = 512
NK = 4            # conv output channels
NT = 3            # iterations
NIMG = 4          # images per core
NCORES = 8
BLK = 32          # out rows per block
RHO = 36          # input rows per block window
KA = 3 * RHO      # pass-A contraction (q = -2..0)
KB = 2 * RHO      # pass-B contraction (q = +1..+2)
ROW = NIMG * 516              # one scratch row: 4 images interleaved
PP = 4 * ROW                  # xb elements per partition (v, im, 516)
XP = 4 * PP                   # xrep elements per partition (u, v, im, 516)
SCR_ROWS = 133                # 128 + 2 ghost rows each side + 1 pad
                              # (q-shifted gather reads a few elements
                              #  past row 131; pad row stays zero)


def _build_C(Wt, R_rows):
    """Effective 5x5 stencil with row masks. Wt [4,8,3,3] float64."""
    nR = len(R_rows)
    C = np.zeros((NK, 5, 5, nR))
    CL = np.zeros((NK, 5, nR))
    CR = np.zeros((NK, 5, nR))
    R = np.asarray(R_rows)
    for c, (di, dj) in enumerate(SHIFTS):
        for u in (-1, 0, 1):
            p = u + di
            m = ((R + u >= 0) & (R + u < H) & (R + p >= 0) & (R + p < H))
            for v in (-1, 0, 1):
                q = v + dj
                w = Wt[:, c, u + 1, v + 1]
                C[:, p + 2, q + 2, :] += w[:, None] * m[None, :]
                if q == 0 and (v, dj) == (-1, 1):
                    CL[:, p + 2, :] -= w[:, None] * m[None, :]
                if q == 0 and (v, dj) == (1, -1):
                    CR[:, p + 2, :] -= w[:, None] * m[None, :]
    return C, CL, CR


def _build_lhsT(Wt, b):
    """lhsT_A [108,128], lhsT_B [72,128], corrL/corrR [108,128] for block b."""
    C, CL, CR = _build_C(Wt, [BLK * b + r for r in range(BLK)])
    A = np.zeros((KA, 128))
    Bm = np.zeros((KB, 128))
    cl = np.zeros((KA, 128))
    cr = np.zeros((KA, 128))
    for r in range(BLK):
        for rho in range(RHO):
            p = rho - 2 - r
            if not -2 <= p <= 2:
                continue
            for k in range(NK):
                for qa in range(3):
                    A[qa * RHO + rho, 32 * k + r] = C[k, p + 2, qa, r]
                for qb in range(2):
                    Bm[qb * RHO + rho, 32 * k + r] = C[k, p + 2, qb + 3, r]
                cl[2 * RHO + rho, 32 * k + r] = CL[k, p + 2, r]
                cr[2 * RHO + rho, 32 * k + r] = CR[k, p + 2, r]
    return A, Bm, cl, cr


def _build_graph(bf, af):
    from contextlib import ExitStack
    import concourse.bass as bass
    import concourse.tile as tile
    from concourse import mybir
    from concourse.ap import AP

    nc = bass.Bass()
    bf16 = mybir.dt.bfloat16
    f32 = mybir.dt.float32
    LR = mybir.ActivationFunctionType.Prelu

    scrs = [nc.declare_dram_parameter(f"xscr{v}", [SCR_ROWS, NIMG, 516],
                                      bf16, isOutput=False)
            for v in range(4)]
    # weights packed: [128, NT, 3var, 4 kinds(A,B,cl,cr), 128] bf16
    w_ext = nc.declare_dram_parameter("wts", [128, NT, 3, 4, 128], bf16,
                                      isOutput=False)
    red_ext = nc.declare_dram_parameter("red", [128, 32], bf16, isOutput=False)
    bias_ext = nc.declare_dram_parameter("biasv", [128, NT], f32,
                                         isOutput=False)
    out_ext = nc.declare_dram_parameter("out", [4, 128, NIMG, 516], bf16,
                                        isOutput=True)

    use_bias = bool(np.any(bf))

    with tile.TileContext(nc) as tc:
        with ExitStack() as ctx:
            persist = ctx.enter_context(tc.tile_pool(name="persist", bufs=1))
            ppool = ctx.enter_context(tc.tile_pool(name="pt", bufs=3))
            p2pool = ctx.enter_context(tc.tile_pool(name="p2", bufs=2))
            pacc_pool = ctx.enter_context(
                tc.tile_pool(name="pa", bufs=3, space="PSUM"))
            sacc_pool = ctx.enter_context(
                tc.tile_pool(name="sa", bufs=2, space="PSUM"))

            wts = persist.tile([128, NT, 3, 4, 128], bf16, tag="wts")
            red = persist.tile([128, 32], bf16, tag="red")
            biasv = persist.tile([128, NT], f32, tag="biasv")
            nc.sync.dma_start(out=wts, in_=w_ext[:, :, :, :, :])
            nc.sync.dma_start(out=red, in_=red_ext[:, :])
            nc.sync.dma_start(out=biasv, in_=bias_ext[:, :])

            xb = persist.tile([128, 4, NIMG, 516], bf16, tag="xb")
            xrepA = persist.tile([KA, 4, 4, NIMG, 516], bf16, tag="xrepA")
            xrepB = persist.tile([KB, 4, 4, NIMG, 516], bf16, tag="xrepB")

            # All gathers go on the sync (SP) HWDGE ring: the SP stream has
            # no compute work, so their long semaphore waits can't
            # head-of-line-block anything.  Writebacks go on the scalar
            # (ACT) ring -- their waits (the just-issued DVE subs) are
            # short, and keeping them off SP lets gathers flow.
            def gather(v, us):
                """xrep[(q,rho), u, v, im, cg] = scr_v[32u+rho, im, cg+coff+q]
                for both kinds; one 2-dim DMA per (kind, u, q) -- plain
                [rows x run] shapes spray descriptors across all 16 SDMA
                engines, fancier shapes collapse onto 2-3."""
                for u in us:
                    for xrep, nsh, coff in ((xrepA, 3, 0), (xrepB, 2, 3)):
                        for q in range(nsh):
                            src = AP(scrs[v], (32 * u) * ROW + coff + q,
                                     [[ROW, RHO], [1, ROW]])
                            dst = AP(xrep.tensor,
                                     xrep.offset + (q * RHO) * XP
                                     + u * PP + v * ROW,
                                     [[XP, RHO], [1, ROW]])
                            nc.sync.dma_start(out=dst, in_=src)

            def writeback(v, t):
                src = AP(xb.tensor, xb.offset + v * ROW, [[PP, 128], [1, ROW]])
                if t == NT - 1:
                    dst = AP(out_ext, v * 128 * ROW, [[ROW, 128], [1, ROW]])
                    nc.scalar.dma_start(out=dst, in_=src)
                    return
                dst = AP(scrs[v], 2 * ROW, [[ROW, 128], [1, ROW]])
                nc.scalar.dma_start(out=dst, in_=src)
                if v > 0:     # rows 128v..128v+1 -> scr_{v-1}[130:132]
                    s = AP(xb.tensor, xb.offset + v * ROW, [[PP, 2], [1, ROW]])
                    d = AP(scrs[v - 1], 130 * ROW, [[ROW, 2], [1, ROW]])
                    nc.scalar.dma_start(out=d, in_=s)
                if v < 3:     # rows 128v+126..127 -> scr_{v+1}[0:2]
                    s = AP(xb.tensor, xb.offset + 126 * PP + v * ROW,
                           [[PP, 2], [1, ROW]])
                    d = AP(scrs[v + 1], 0, [[ROW, 2], [1, ROW]])
                    nc.scalar.dma_start(out=d, in_=s)

            # prologue: t=0 gathers + xb load, all from host-filled scratch
            for v in range(4):
                gather(v, range(4))
                src = AP(scrs[v], 2 * ROW, [[ROW, 128], [1, ROW]])
                dst = AP(xb.tensor, xb.offset + v * ROW, [[PP, 128], [1, ROW]])
                nc.sync.dma_start(out=dst, in_=src)

            for t in range(NT):
                alpha = float(np.sqrt(af[t]))
                for v in range(4):
                    for im in range(NIMG):
                        p2 = p2pool.tile([128, 4, 512], bf16, tag="p2")
                        for pairu in (0, 1):
                            acc = pacc_pool.tile([128, 2, 512],
                                                 mybir.dt.float32, tag="acc")
                            for j in (0, 1):
                                u = 2 * pairu + j
                                var = 0 if (v == 0 and u == 0) else \
                                    (2 if (v == 3 and u == 3) else 1)
                                nc.tensor.matmul(
                                    acc[:, j, :], wts[0:KA, t, var, 0, :],
                                    xrepA[:, u, v, im, 0:512],
                                    start=True, stop=False)
                                nc.tensor.matmul(
                                    acc[:, j, :], wts[0:KB, t, var, 1, :],
                                    xrepB[:, u, v, im, 0:512],
                                    start=False, stop=False)
                                nc.tensor.matmul(
                                    acc[:, j, 0:1], wts[0:KA, t, var, 2, :],
                                    xrepA[:, u, v, im, 0:1],
                                    start=False, stop=False)
                                nc.tensor.matmul(
                                    acc[:, j, 511:512], wts[0:KA, t, var, 3, :],
                                    xrepA[:, u, v, im, 511:512],
                                    start=False, stop=True)
                            ptile = ppool.tile([128, 2, 512], bf16, tag="ptile")
                            if use_bias:
                                nc.scalar.activation(
                                    out=ptile, in_=acc, func=LR,
                                    bias=biasv[:, t:t + 1], scale=0.5,
                                    alpha=alpha)
                            else:
                                nc.scalar.activation(
                                    out=ptile, in_=acc, func=LR,
                                    bias=0.0, scale=0.5, alpha=alpha)
                            nc.vector.tensor_mul(
                                p2[:, 2 * pairu:2 * pairu + 2, :],
                                ptile, ptile)
                        sacc = sacc_pool.tile([128, 512], mybir.dt.float32,
                                              tag="sacc")
                        for u in range(4):
                            nc.tensor.matmul(
                                sacc[32 * u:32 * u + 32, :], red, p2[:, u, :],
                                start=True, stop=True,
                                tile_position=(0, 32 * u))
                        nc.vector.tensor_sub(
                            xb[:, v, im, 2:514], xb[:, v, im, 2:514], sacc)
                    writeback(v, t)
                    if t < NT - 1:
                        # u<=2 windows of tile v need only wb(v-1..v), the
                        # u=3 window also needs wb(v+1): emit each gather
                        # DMA at the moment its last dependency is issued,
                        # so the SP FIFO drains in dependency order.
                        gather(v, (0, 1, 2))
                        if v >= 1:
                            gather(v - 1, (3,))
                        if v == 3:
                            gather(3, (3,))

    _split_multiwait_drains(nc)
    return nc


def _split_multiwait_drains(nc):
    """Walrus workaround: this compiler build only accepts one sem-wait per
    instruction; peel extras onto injected same-engine NoOps placed just
    before (engine streams run in program order, so semantics are equal)."""
    from concourse import mybir
    import bass_rust

    for f in nc.m.functions:
        for bb in f.blocks:
            idx = 0
            while idx < len(bb.instructions):
                inst = bb.instructions[idx]
                si = getattr(inst, "sync_info", None)
                if si is not None and si.on_wait and len(si.on_wait) > 1:
                    waits = list(si.on_wait)
                    upd = list(si.on_update) if si.on_update else []
                    for j, w in enumerate(waits[:-1]):
                        nop = mybir.InstNoOp(
                            name=f"{inst.name}-wsplit{j}", ins=[], outs=[])
                        nop.engine = inst.engine
                        nop.sync_info = bass_rust.SyncInfo(
                            on_wait=[w], on_update=[])
                        nc.register_instruction(nop, overwrite=True)
                        bb.instructions.insert(idx, nop)
                        idx += 1
                    inst.sync_info = bass_rust.SyncInfo(
                        on_wait=[waits[-1]], on_update=upd)
                idx += 1


def kernel(x, W, b, a):
    from concourse.bass_utils import run_bass_kernel_spmd

    x = np.asarray(x)
    Wf = np.asarray(W, dtype=np.float64)
    bfv = np.asarray(b, dtype=np.float64)
    af = np.asarray(a, dtype=np.float64)

    # weights: [128, NT, 3var, 4kinds, 128] (partition = contraction index)
    wts = np.zeros((128, NT, 3, 4, 128), np.float64)
    for t in range(NT):
        for vi, blk in enumerate((0, 1, 15)):
            A, Bm, cl, cr = _build_lhsT(Wf[t], blk)
            wts[0:KA, t, vi, 0, :] = A
            wts[0:KB, t, vi, 1, :] = Bm
            wts[0:KA, t, vi, 2, :] = cl
            wts[0:KA, t, vi, 3, :] = cr
    wts = wts.astype(ml_dtypes.bfloat16)

    red = np.zeros((128, 32), ml_dtypes.bfloat16)
    for k in range(NK):
        for r in range(BLK):
            red[32 * k + r, r] = 1.0
    biasv = np.zeros((128, NT), np.float32)
    for t in range(NT):
        biasv[:, t] = np.repeat(0.5 * bfv[t], BLK)

    nc = _build_graph(bfv, af)

    xall = x[:, 0].astype(ml_dtypes.bfloat16)   # [32, 512, 512]
    in_maps = []
    for core in range(NCORES):
        shard = xall[core * NIMG:(core + 1) * NIMG]   # [4, 512, 512]
        im = {"wts": wts, "red": red, "biasv": biasv}
        for v in range(4):
            scr = np.zeros((SCR_ROWS, NIMG, 516), dtype=ml_dtypes.bfloat16)
            lo = 2 if v == 0 else 0
            hi = 130 if v == 3 else 132
            scr[lo:hi, :, 2:514] = \
                shard[:, 128 * v - 2 + lo:128 * v - 2 + hi, :] \
                .transpose(1, 0, 2)
            im[f"xscr{v}"] = scr
        in_maps.append(im)
    res = run_bass_kernel_spmd(nc, in_maps, list(range(NCORES)))
    global LAST_RESULT
    LAST_RESULT = res
    out = np.empty((32, H, 512), dtype=np.float32)
    for core in range(NCORES):
        ot = np.asarray(res.results[core]["out"], dtype=np.float32)
        for v in range(4):
            out[core * NIMG:(core + 1) * NIMG, 128 * v:128 * v + 128, :] = \
                ot[v, :, :, 2:514].transpose(1, 0, 2)
    return out[:, None, :, :].astype(x.dtype)


LAST_RESULT = None


# revision 15
# speedup vs baseline: 2.1265x; 1.1831x over previous
"""Trainium2 Bass kernel for nn_AD_F_56384330662393 (dense_cnn, 3-iter
diffusion).  Data-parallel over batch: 32 images -> 8 cores x 4 images.

Scheme C ("q-folded im2col") with DRAM-routed halo gathers:
per image, 16 blocks of 32 output rows.  Per block ONE PSUM accumulation
group of 4 matmuls:
  - A: lhsT [108,128] (3 col-shifts x 36 rows folded into K), all 4 conv
    channels packed into M=128 (out partition = 32k + r)
  - B: lhsT [72,128] (2 more col-shifts)
  - corrL/corrR: N=1 column-boundary corrections (feature-pad semantics).
The rhs operands are q-replicated 36-row windows (xrep), gathered from a
row-linear DRAM scratch (x rows interleaved over images at stride 516,
2 ghost rows/cols zeroed host-side).  DRAM-side strides absorb the
(q: +1 col, rho: +1 row) pattern, so one DMA per (kind,u,v) rebuilds a
whole 108/72-partition slab and the descriptors spread across all 16
SDMA engines (SBUF->SBUF gathers collapse onto 1 engine pair; DRAM
round-trip is ~3x faster end-to-end).

Pipelined v-major loop: after tile v's 4 images are updated, xb[:,v] is
written back to the per-v scratch (plus 2-row halo strips to v+-1) and
the iteration-(t+1) gathers for tile v-1 are issued immediately, so all
gather/writeback DMA runs in the shadow of the next tiles' matmuls.
Gathers/writebacks alternate between the two HWDGE rings (sync/scalar).

PReLU^2 via one ScalarE Prelu over 2 banks then DVE square; channel
reduction S = sum_k p_k^2 via 4 col-tiled 0/1 matmuls (tile_position)
into one PSUM bank; one DVE sub per 128-row column-tile updates x.

x layout: [128, v, im, 516] bf16 in SBUF (image row r = partition r%128,
col-tile v = r//128); scratch_v in DRAM: [132, im, 516] rows 128v-2 ..
128v+129.  x/weights/output shipped bf16.
"""
import numpy as np
import ml_dtypes

SHIFTS = [(-1, 0), (1, 0), (0, -1), (0, 1), (-1, -1), (-1, 1), (1, -1), (1, 1)]
H = 512
NK = 4            # conv output channels
NT = 3            # iterations
NIMG = 4          # images per core
NCORES = 8
BLK = 32          # out rows per block
RHO = 36          # input rows per block window
KA = 3 * RHO      # pass-A contraction (q = -2..0)
KB = 2 * RHO      # pass-B contraction (q = +1..+2)
ROW = NIMG * 516              # one scratch row: 4 images interleaved
PP = 4 * ROW                  # xb elements per partition (v, im, 516)
XP = 4 * PP                   # xrep elements per partition (u, v, im, 516)
SCR_ROWS = 133                # 128 + 2 ghost rows each side + 1 pad
                              # (q-shifted gather reads a few elements
                              #  past row 131; pad row stays zero)


def _build_C(Wt, R_rows):
    """Effective 5x5 stencil with row masks. Wt [4,8,3,3] float64."""
    nR = len(R_rows)
    C = np.zeros((NK, 5, 5, nR))
    CL = np.zeros((NK, 5, nR))
    CR = np.zeros((NK, 5, nR))
    R = np.asarray(R_rows)
    for c, (di, dj) in enumerate(SHIFTS):
        for u in (-1, 0, 1):
            p = u + di
            m = ((R + u >= 0) & (R + u < H) & (R + p >= 0) & (R + p < H))
            for v in (-1, 0, 1):
                q = v + dj
                w = Wt[:, c, u + 1, v + 1]
                C[:, p + 2, q + 2, :] += w[:, None] * m[None, :]
                if q == 0 and (v, dj) == (-1, 1):
                    CL[:, p + 2, :] -= w[:, None] * m[None, :]
                if q == 0 and (v, dj) == (1, -1):
                    CR[:, p + 2, :] -= w[:, None] * m[None, :]
    return C, CL, CR


def _build_lhsT(Wt, b):
    """lhsT_A [108,128], lhsT_B [72,128], corrL/corrR [108,128] for block b."""
    C, CL, CR = _build_C(Wt, [BLK * b + r for r in range(BLK)])
    A = np.zeros((KA, 128))
    Bm = np.zeros((KB, 128))
    cl = np.zeros((KA, 128))
    cr = np.zeros((KA, 128))
    for r in range(BLK):
        for rho in range(RHO):
            p = rho - 2 - r
            if not -2 <= p <= 2:
                continue
            for k in range(NK):
                for qa in range(3):
                    A[qa * RHO + rho, 32 * k + r] = C[k, p + 2, qa, r]
                for qb in range(2):
                    Bm[qb * RHO + rho, 32 * k + r] = C[k, p + 2, qb + 3, r]
                cl[2 * RHO + rho, 32 * k + r] = CL[k, p + 2, r]
                cr[2 * RHO + rho, 32 * k + r] = CR[k, p + 2, r]
    return A, Bm, cl, cr


def _build_graph(bf, af):
    from contextlib import ExitStack
    import concourse.bass as bass
    import concourse.tile as tile
    from concourse import mybir
    from concourse.ap import AP

    nc = bass.Bass()
    bf16 = mybir.dt.bfloat16
    f32 = mybir.dt.float32
    LR = mybir.ActivationFunctionType.Prelu

    scrs = [nc.declare_dram_parameter(f"xscr{v}", [SCR_ROWS, NIMG, 516],
                                      bf16, isOutput=False)
            for v in range(4)]
    # weights packed: [128, NT, 3var, 4 kinds(A,B,cl,cr), 128] bf16
    w_ext = nc.declare_dram_parameter("wts", [128, NT, 3, 4, 128], bf16,
                                      isOutput=False)
    red_ext = nc.declare_dram_parameter("red", [128, 32], bf16, isOutput=False)
    bias_ext = nc.declare_dram_parameter("biasv", [128, NT], f32,
                                         isOutput=False)
    out_ext = nc.declare_dram_parameter("out", [4, 128, NIMG, 516], bf16,
                                        isOutput=True)

    use_bias = bool(np.any(bf))

    with tile.TileContext(nc) as tc:
        with ExitStack() as ctx:
            persist = ctx.enter_context(tc.tile_pool(name="persist", bufs=1))
            ppool = ctx.enter_context(tc.tile_pool(name="pt", bufs=3))
            p2pool = ctx.enter_context(tc.tile_pool(name="p2", bufs=2))
            pacc_pool = ctx.enter_context(
                tc.tile_pool(name="pa", bufs=3, space="PSUM"))
            sacc_pool = ctx.enter_context(
                tc.tile_pool(name="sa", bufs=2, space="PSUM"))

            wts = persist.tile([128, NT, 3, 4, 128], bf16, tag="wts")
            red = persist.tile([128, 32], bf16, tag="red")
            biasv = persist.tile([128, NT], f32, tag="biasv")
            nc.sync.dma_start(out=wts, in_=w_ext[:, :, :, :, :])
            nc.sync.dma_start(out=red, in_=red_ext[:, :])
            nc.sync.dma_start(out=biasv, in_=bias_ext[:, :])

            xb = persist.tile([128, 4, NIMG, 516], bf16, tag="xb")
            xrepA = persist.tile([KA, 4, 4, NIMG, 516], bf16, tag="xrepA")
            xrepB = persist.tile([KB, 4, 4, NIMG, 516], bf16, tag="xrepB")

            # All gathers go on the sync (SP) HWDGE ring: the SP stream has
            # no compute work, so their long semaphore waits can't
            # head-of-line-block anything.  Writebacks go on the scalar
            # (ACT) ring -- their waits (the just-issued DVE subs) are
            # short, and keeping them off SP lets gathers flow.
            def gather(v, us):
                """xrep[(q,rho), u, v, im, cg] = scr_v[32u+rho, im, cg+coff+q]
                for both kinds; one 2-dim DMA per (kind, u, q) -- plain
                [rows x run] shapes spray descriptors across all 16 SDMA
                engines, fancier shapes collapse onto 2-3."""
                for u in us:
                    for xrep, nsh, coff, eng in ((xrepA, 3, 0, nc.sync),
                                                 (xrepB, 2, 3, nc.gpsimd)):
                        for q in range(nsh):
                            src = AP(scrs[v], (32 * u) * ROW + coff + q,
                                     [[ROW, RHO], [1, ROW]])
                            dst = AP(xrep.tensor,
                                     xrep.offset + (q * RHO) * XP
                                     + u * PP + v * ROW,
                                     [[XP, RHO], [1, ROW]])
                            eng.dma_start(out=dst, in_=src)

            def writeback(v, t):
                src = AP(xb.tensor, xb.offset + v * ROW, [[PP, 128], [1, ROW]])
                if t == NT - 1:
                    dst = AP(out_ext, v * 128 * ROW, [[ROW, 128], [1, ROW]])
                    nc.scalar.dma_start(out=dst, in_=src)
                    return
                dst = AP(scrs[v], 2 * ROW, [[ROW, 128], [1, ROW]])
                nc.scalar.dma_start(out=dst, in_=src)
                if v > 0:     # rows 128v..128v+1 -> scr_{v-1}[130:132]
                    s = AP(xb.tensor, xb.offset + v * ROW, [[PP, 2], [1, ROW]])
                    d = AP(scrs[v - 1], 130 * ROW, [[ROW, 2], [1, ROW]])
                    nc.scalar.dma_start(out=d, in_=s)
                if v < 3:     # rows 128v+126..127 -> scr_{v+1}[0:2]
                    s = AP(xb.tensor, xb.offset + 126 * PP + v * ROW,
                           [[PP, 2], [1, ROW]])
                    d = AP(scrs[v + 1], 0, [[ROW, 2], [1, ROW]])
                    nc.scalar.dma_start(out=d, in_=s)

            # prologue: t=0 gathers + xb load, all from host-filled scratch
            for v in range(4):
                gather(v, range(4))
                src = AP(scrs[v], 2 * ROW, [[ROW, 128], [1, ROW]])
                dst = AP(xb.tensor, xb.offset + v * ROW, [[PP, 128], [1, ROW]])
                nc.scalar.dma_start(out=dst, in_=src)

            for t in range(NT):
                alpha = float(np.sqrt(af[t]))
                for v in range(4):
                    for im in range(NIMG):
                        p2 = p2pool.tile([128, 4, 512], bf16, tag="p2")
                        for pairu in (0, 1):
                            acc = pacc_pool.tile([128, 2, 512],
                                                 mybir.dt.float32, tag="acc")
                            for j in (0, 1):
                                u = 2 * pairu + j
                                var = 0 if (v == 0 and u == 0) else \
                                    (2 if (v == 3 and u == 3) else 1)
                                nc.tensor.matmul(
                                    acc[:, j, :], wts[0:KA, t, var, 0, :],
                                    xrepA[:, u, v, im, 0:512],
                                    start=True, stop=False)
                                nc.tensor.matmul(
                                    acc[:, j, :], wts[0:KB, t, var, 1, :],
                                    xrepB[:, u, v, im, 0:512],
                                    start=False, stop=False)
                                nc.tensor.matmul(
                                    acc[:, j, 0:1], wts[0:KA, t, var, 2, :],
                                    xrepA[:, u, v, im, 0:1],
                                    start=False, stop=False)
                                nc.tensor.matmul(
                                    acc[:, j, 511:512], wts[0:KA, t, var, 3, :],
                                    xrepA[:, u, v, im, 511:512],
                                    start=False, stop=True)
                            ptile = ppool.tile([128, 2, 512], bf16, tag="ptile")
                            if use_bias:
                                nc.scalar.activation(
                                    out=ptile, in_=acc, func=LR,
                                    bias=biasv[:, t:t + 1], scale=0.5,
                                    alpha=alpha)
                            else:
                                nc.scalar.activation(
                                    out=ptile, in_=acc, func=LR,
                                    bias=0.0, scale=0.5, alpha=alpha)
                            nc.vector.tensor_mul(
                                p2[:, 2 * pairu:2 * pairu + 2, :],
                                ptile, ptile)
                        sacc = sacc_pool.tile([128, 512], mybir.dt.float32,
                                              tag="sacc")
                        for u in range(4):
                            nc.tensor.matmul(
                                sacc[32 * u:32 * u + 32, :], red, p2[:, u, :],
                                start=True, stop=True,
                                tile_position=(0, 32 * u))
                        nc.vector.tensor_sub(
                            xb[:, v, im, 2:514], xb[:, v, im, 2:514], sacc)
                    writeback(v, t)
                    if t < NT - 1:
                        # u<=2 windows of tile v need only wb(v-1..v), the
                        # u=3 window also needs wb(v+1): emit each gather
                        # DMA at the moment its last dependency is issued,
                        # so the SP FIFO drains in dependency order.
                        gather(v, (0, 1, 2))
                        if v >= 1:
                            gather(v - 1, (3,))
                        if v == 3:
                            gather(3, (3,))

    _split_multiwait_drains(nc)
    return nc


def _split_multiwait_drains(nc):
    """Walrus workaround: this compiler build only accepts one sem-wait per
    instruction; peel extras onto injected same-engine NoOps placed just
    before (engine streams run in program order, so semantics are equal)."""
    from concourse import mybir
    import bass_rust

    for f in nc.m.functions:
        for bb in f.blocks:
            idx = 0
            while idx < len(bb.instructions):
                inst = bb.instructions[idx]
                si = getattr(inst, "sync_info", None)
                if si is not None and si.on_wait and len(si.on_wait) > 1:
                    waits = list(si.on_wait)
                    upd = list(si.on_update) if si.on_update else []
                    for j, w in enumerate(waits[:-1]):
                        nop = mybir.InstNoOp(
                            name=f"{inst.name}-wsplit{j}", ins=[], outs=[])
                        nop.engine = inst.engine
                        nop.sync_info = bass_rust.SyncInfo(
                            on_wait=[w], on_update=[])
                        nc.register_instruction(nop, overwrite=True)
                        bb.instructions.insert(idx, nop)
                        idx += 1
                    inst.sync_info = bass_rust.SyncInfo(
                        on_wait=[waits[-1]], on_update=upd)
                idx += 1


def kernel(x, W, b, a):
    from concourse.bass_utils import run_bass_kernel_spmd

    x = np.asarray(x)
    Wf = np.asarray(W, dtype=np.float64)
    bfv = np.asarray(b, dtype=np.float64)
    af = np.asarray(a, dtype=np.float64)

    # weights: [128, NT, 3var, 4kinds, 128] (partition = contraction index)
    wts = np.zeros((128, NT, 3, 4, 128), np.float64)
    for t in range(NT):
        for vi, blk in enumerate((0, 1, 15)):
            A, Bm, cl, cr = _build_lhsT(Wf[t], blk)
            wts[0:KA, t, vi, 0, :] = A
            wts[0:KB, t, vi, 1, :] = Bm
            wts[0:KA, t, vi, 2, :] = cl
            wts[0:KA, t, vi, 3, :] = cr
    wts = wts.astype(ml_dtypes.bfloat16)

    red = np.zeros((128, 32), ml_dtypes.bfloat16)
    for k in range(NK):
        for r in range(BLK):
            red[32 * k + r, r] = 1.0
    biasv = np.zeros((128, NT), np.float32)
    for t in range(NT):
        biasv[:, t] = np.repeat(0.5 * bfv[t], BLK)

    nc = _build_graph(bfv, af)

    xall = x[:, 0].astype(ml_dtypes.bfloat16)   # [32, 512, 512]
    in_maps = []
    for core in range(NCORES):
        shard = xall[core * NIMG:(core + 1) * NIMG]   # [4, 512, 512]
        im = {"wts": wts, "red": red, "biasv": biasv}
        for v in range(4):
            scr = np.zeros((SCR_ROWS, NIMG, 516), dtype=ml_dtypes.bfloat16)
            lo = 2 if v == 0 else 0
            hi = 130 if v == 3 else 132
            scr[lo:hi, :, 2:514] = \
                shard[:, 128 * v - 2 + lo:128 * v - 2 + hi, :] \
                .transpose(1, 0, 2)
            im[f"xscr{v}"] = scr
        in_maps.append(im)
    res = run_bass_kernel_spmd(nc, in_maps, list(range(NCORES)))
    global LAST_RESULT
    LAST_RESULT = res
    out = np.empty((32, H, 512), dtype=np.float32)
    for core in range(NCORES):
        ot = np.asarray(res.results[core]["out"], dtype=np.float32)
        for v in range(4):
            out[core * NIMG:(core + 1) * NIMG, 128 * v:128 * v + 128, :] = \
                ot[v, :, :, 2:514].transpose(1, 0, 2)
    return out[:, None, :, :].astype(x.dtype)


LAST_RESULT = None
